# revision 3
# baseline (speedup 1.0000x reference)
"""PolyConv (5-tap graph Laplacian polynomial) on 8 Trainium2 NeuronCores.

Reference computation:
    deg = segment_sum(ones, dst); dinv = max(deg,1)^-1/2
    L(f) = f - dinv * segment_sum((f*dinv)[src], dst)
    h = sum_k THETA[k] * L^k(feat)

Sharding: nodes (and edges, by dst) are partitioned across the 8 cores.
Each core owns a contiguous node shard. Per iteration it computes
g = f * dinv on its shard, AllGathers g into a replicated DRAM table,
gathers g[src[e]] for its dst-local edges with gpsimd.dma_gather
(4 sub-tables to satisfy the int16 index range), and scatter-adds via
one-hot matmuls on the PE into a PSUM-resident per-shard aggregate.
Host-side work is layout only: shard/sort/pad the edge list into a
core-uniform (bucket, window) block grid and concat core outputs.
"""
import numpy as np

import concourse.bacc as bacc
import concourse.mybir as mybir
import concourse.tile as tile
from concourse.bass_utils import run_bass_kernel_spmd

P = 128
NC = 8
NSUB = 4          # dma_gather subtables (int16 idx < 32768)
GCHUNK = 8192     # idxs per dma_gather instruction
THETA = [1.0, -0.5, 0.25, -0.125, 0.0625]
K_ITERS = len(THETA) - 1
F32 = mybir.dt.float32

_kernel_cache = {}


def _layout(src, dst, n_nodes):
    """Core-uniform (bucket, window) block grid.

    Returns (schedule, per_core) where schedule is identical across cores:
      blk_win[j]  : dst window of block j
      blk_bkt[j]  : gather subtable bucket of block j
    and per_core[c] provides idx_local / dst_lo slot tensors.
    """
    shard = n_nodes // NC
    shard_pad = -(-shard // P) * P
    nw = shard_pad // P
    ntab = shard_pad * NC
    sub = ntab // NSUB
    assert sub <= 32768 and ntab % NSUB == 0

    owner = dst // shard
    tsh = src // shard
    trow = tsh * shard_pad + (src - tsh * shard)      # g-table row of src
    wloc = (dst - owner * shard) // P                 # dst window in shard
    dloc = (dst - owner * shard) % P                  # dst lane in window
    bkt = trow // sub

    # counts per (core, bucket, window)
    cnt = np.zeros((NC, NSUB, nw), np.int64)
    np.add.at(cnt, (owner, bkt, wloc), 1)
    blocks = -(-np.max(cnt, axis=0) // P)             # [NSUB, nw] uniform grid
    blocks = np.maximum(blocks, 0)

    blk_win, blk_bkt = [], []
    for b in range(NSUB):
        for w in range(nw):
            blk_win += [w] * int(blocks[b, w])
            blk_bkt += [b] * int(blocks[b, w])
    blk_win = np.array(blk_win, np.int64)
    blk_bkt = np.array(blk_bkt, np.int64)
    nblk = len(blk_win)

    # slot offset of each (bucket, window) group
    goff = np.zeros((NSUB, nw), np.int64)
    run = 0
    for b in range(NSUB):
        for w in range(nw):
            goff[b, w] = run
            run += int(blocks[b, w]) * P
    nslot = run

    per_core = []
    order = np.lexsort((wloc, bkt, owner))
    o_owner, o_bkt, o_w = owner[order], bkt[order], wloc[order]
    o_trow, o_dloc = trow[order], dloc[order]
    # position within (core,bucket,window) group
    gid = ((o_owner * NSUB + o_bkt) * nw + o_w)
    starts = np.searchsorted(gid, np.arange(NC * NSUB * nw))
    pos = np.arange(len(gid)) - starts[gid]
    slot = goff[o_bkt, o_w] + pos
    for c in range(NC):
        m = o_owner == c
        idx_local = np.zeros(nslot, np.int64)
        dst_lo = np.full(nslot, -1.0, np.float32)
        idx_local[slot[m]] = o_trow[m] - o_bkt[m] * sub
        dst_lo[slot[m]] = o_dloc[m]
        per_core.append((idx_local, dst_lo))

    sched = dict(shard=shard, shard_pad=shard_pad, nw=nw, ntab=ntab, sub=sub,
                 blk_win=blk_win, blk_bkt=blk_bkt, nblk=nblk, nslot=nslot,
                 blocks=blocks)
    return sched, per_core


def _wrap_idx16(idx_local, sched):
    """Per-chunk 16-wrap + replicate to 128 partitions -> [128, nslot/16]."""
    cols = []
    s = 0
    blocks = sched["blocks"]
    for b in range(NSUB):
        nb = int(blocks[b].sum()) * P
        e = s + nb
        for c0 in range(s, e, GCHUNK):
            cn = min(GCHUNK, e - c0)
            w = idx_local[c0:c0 + cn].reshape(cn // 16, 16).T.astype(np.int16)
            cols.append(w)
        s = e
    w = np.concatenate(cols, axis=1) if cols else np.zeros((16, 0), np.int16)
    return np.tile(w, (8, 1))


def _build(sched, d_feat):
    nw = sched["nw"]
    ntab = sched["ntab"]
    sub = sched["sub"]
    shard_pad = sched["shard_pad"]
    blk_win = sched["blk_win"]
    nblk = sched["nblk"]
    nslot = sched["nslot"]
    blocks = sched["blocks"]
    D = d_feat
    ELEM = 2 * D                      # g-table row f32 (2*D*4 = 256B for D=32)
    WPB = 512 // D                    # windows per PSUM bank (16 for D=32)

    # first/last block index per PSUM bank of the agg region
    bank_of = blk_win // WPB
    first_of_bank, last_of_bank = {}, {}
    for j in range(nblk):
        bb = int(bank_of[j])
        if bb not in first_of_bank:
            first_of_bank[bb] = j
        last_of_bank[bb] = j

    nc = bacc.Bacc(None, target_bir_lowering=False,
                   dynamic_dma_scratch_size=65536)

    feat_in = nc.declare_dram_parameter("feat", [shard_pad, D], F32, isOutput=False)
    idx16 = nc.declare_dram_parameter("idx16", [P, nslot // 16], mybir.dt.int16, isOutput=False)
    dstlo_in = nc.declare_dram_parameter("dstlo", [P, nblk], F32, isOutput=False)
    iota_in = nc.declare_dram_parameter("iota", [P, P], F32, isOutput=False)
    h_out = nc.declare_dram_parameter("h", [shard_pad, D], F32, isOutput=True)

    g_shard = nc.dram_tensor("g_shard", [shard_pad, ELEM], F32)
    g_table = nc.dram_tensor("g_table", [ntab, ELEM], F32, addr_space="Shared")
    core_ids = list(range(NC))

    with tile.TileContext(nc) as tc:
        with tc.tile_pool(name="sb", bufs=1) as sb, \
             tc.tile_pool(name="vp", bufs=2) as vp, \
             tc.tile_pool(name="ohp", bufs=4) as ohp, \
             tc.tile_pool(name="pp", bufs=1, space="PSUM") as pp:

            f_t = sb.tile([P, nw * D], F32)
            h_t = sb.tile([P, nw * D], F32)
            scr_t = sb.tile([P, nw * D], F32)
            g_sb = sb.tile([P, nw * ELEM], F32)
            dinv_t = sb.tile([P, nw], F32)
            iota_t = sb.tile([P, P], F32)
            dstlo_t = sb.tile([P, nblk], F32)
            idx_t = sb.tile([P, nslot // 16], mybir.dt.int16)
            ones_t = sb.tile([P, 1], F32)

            nc.sync.dma_start(out=iota_t[:], in_=iota_in[:, :])
            nc.sync.dma_start(out=dstlo_t[:], in_=dstlo_in[:, :])
            nc.sync.dma_start(out=idx_t[:], in_=idx16[:, :])
            nc.sync.dma_start(
                out=f_t[:].rearrange("p (w d) -> p w d", d=D),
                in_=feat_in[:, :].rearrange("(w p) d -> p w d", p=P))
            nc.vector.memset(ones_t[:], 1.0)
            nc.vector.memset(g_sb[:], 0.0)

            agg_ps = pp.tile([P, nw * D], F32, space="PSUM")
            deg_ps = pp.tile([P, nw], F32, space="PSUM")

            def onehot(j, name):
                oh = ohp.tile([P, P], F32, tag="oh", name=name)
                nc.vector.tensor_scalar(
                    out=oh[:], in0=iota_t[:], scalar1=dstlo_t[:, j:j + 1],
                    scalar2=None, op0=mybir.AluOpType.is_equal)
                return oh

            # ---- degree pass ----
            for j in range(nblk):
                oh = onehot(j, f"ohd{j % 4}")
                nc.tensor.matmul(
                    out=deg_ps[:, blk_win[j]:blk_win[j] + 1],
                    lhsT=oh[:], rhs=ones_t[:],
                    start=(j == 0), stop=(j == nblk - 1))
            degc = sb.tile([P, nw], F32)
            nc.vector.tensor_scalar(out=degc[:], in0=deg_ps[:], scalar1=1.0,
                                    scalar2=None, op0=mybir.AluOpType.max)
            nc.vector.reciprocal(degc[:], degc[:])
            nc.scalar.activation(dinv_t[:], degc[:],
                                 mybir.ActivationFunctionType.Sqrt)

            # h = theta0 * f
            nc.vector.tensor_scalar(out=h_t[:], in0=f_t[:],
                                    scalar1=float(THETA[0]), scalar2=None,
                                    op0=mybir.AluOpType.mult)

            dinv_b = dinv_t[:].rearrange("p (w x) -> p w x", x=1).to_broadcast([P, nw, D])

            for it in range(K_ITERS):
                # g = f * dinv  (columns D..ELEM stay zero)
                nc.vector.tensor_tensor(
                    out=g_sb[:].rearrange("p (w e) -> p w e", e=ELEM)[:, :, 0:D],
                    in0=f_t[:].rearrange("p (w d) -> p w d", d=D),
                    in1=dinv_b, op=mybir.AluOpType.mult)
                nc.sync.dma_start(
                    out=g_shard[:, :].rearrange("(w p) e -> p w e", p=P),
                    in_=g_sb[:].rearrange("p (w e) -> p w e", e=ELEM))
                nc.gpsimd.collective_compute(
                    "AllGather", mybir.AluOpType.bypass,
                    replica_groups=[core_ids],
                    ins=[g_shard[:, :]], outs=[g_table[:, :]])

                # gather + scatter
                s = 0
                j = 0
                ci = 0
                for b in range(NSUB):
                    nb_slots = int(blocks[b].sum()) * P
                    for c0 in range(0, nb_slots, GCHUNK):
                        cn = min(GCHUNK, nb_slots - c0)
                        vals = vp.tile([P, (GCHUNK // P) * ELEM], F32,
                                       tag="vals", name=f"v{it}_{ci % 2}")
                        ci += 1
                        nc.gpsimd.dma_gather(
                            out_ap=vals[:, :(cn // P) * ELEM].rearrange(
                                "p (k e) -> p k e", e=ELEM),
                            in_ap=g_table[b * sub:(b + 1) * sub, :],
                            idxs_ap=idx_t[:, (s + c0) // 16:(s + c0 + cn) // 16],
                            num_idxs=cn, num_idxs_reg=cn, elem_size=ELEM,
                            single_packet=False)
                        for k in range(cn // P):
                            w = int(blk_win[j])
                            bb = int(bank_of[j])
                            oh = onehot(j, f"oha{it}_{j % 4}")
                            nc.tensor.matmul(
                                out=agg_ps[:, w * D:(w + 1) * D], lhsT=oh[:],
                                rhs=vals[:].rearrange(
                                    "p (k e) -> p k e", e=ELEM)[:, k, 0:D],
                                start=(j == first_of_bank[bb]),
                                stop=(j == last_of_bank[bb]))
                            j += 1
                    s += nb_slots

                # f = f - agg * dinv ; h += theta * f
                nc.vector.tensor_tensor(
                    out=scr_t[:].rearrange("p (w d) -> p w d", d=D),
                    in0=agg_ps[:].rearrange("p (w d) -> p w d", d=D),
                    in1=dinv_b, op=mybir.AluOpType.mult)
                nc.vector.tensor_tensor(out=f_t[:], in0=f_t[:], in1=scr_t[:],
                                        op=mybir.AluOpType.subtract)
                nc.vector.tensor_scalar(out=scr_t[:], in0=f_t[:],
                                        scalar1=float(THETA[it + 1]),
                                        scalar2=None, op0=mybir.AluOpType.mult)
                nc.vector.tensor_tensor(out=h_t[:], in0=h_t[:], in1=scr_t[:],
                                        op=mybir.AluOpType.add)

            nc.sync.dma_start(
                out=h_out[:, :].rearrange("(w p) d -> p w d", p=P),
                in_=h_t[:].rearrange("p (w d) -> p w d", d=D))

    nc.finalize()
    return nc


def kernel(feat, src, dst):
    feat = np.ascontiguousarray(np.asarray(feat), dtype=np.float32)
    src = np.asarray(src).astype(np.int64)
    dst = np.asarray(dst).astype(np.int64)
    n_nodes, d_feat = feat.shape

    sched, per_core = _layout(src, dst, n_nodes)
    shard, shard_pad = sched["shard"], sched["shard_pad"]

    key = (n_nodes, d_feat, sched["nblk"], sched["nslot"],
           tuple(sched["blocks"].ravel()[:64]))
    if key not in _kernel_cache:
        _kernel_cache[key] = _build(sched, d_feat)
    nc = _kernel_cache[key]

    iota = np.broadcast_to(np.arange(P, dtype=np.float32), (P, P)).copy()
    in_maps = []
    for c in range(NC):
        idx_local, dst_lo = per_core[c]
        fs = np.zeros((shard_pad, d_feat), np.float32)
        fs[:shard] = feat[c * shard:(c + 1) * shard]
        nb = sched["nblk"]
        in_maps.append({
            "feat": fs,
            "idx16": _wrap_idx16(idx_local, sched),
            "dstlo": dst_lo.reshape(nb, P).T.copy(),
            "iota": iota,
        })

    res = run_bass_kernel_spmd(nc, in_maps, list(range(NC)))
    h = np.empty((n_nodes, d_feat), np.float32)
    for c in range(NC):
        h[c * shard:(c + 1) * shard] = res.results[c]["h"][:shard]
    return h
